# revision 28
# baseline (speedup 1.0000x reference)
import os
import numpy as np

import concourse.bass as bass
import concourse.tile as tile
from concourse import mybir
from concourse.bass_utils import run_bass_kernel_spmd

F32 = mybir.dt.float32
F32R = mybir.dt.float32r
I32 = mybir.dt.int32
AX = mybir.AxisListType
OP = mybir.AluOpType
AF = mybir.ActivationFunctionType

N = 50000
E = 400000
DIM = 16
BOND = 4
RANK = 512
NCORES = 8
NLOC = N // NCORES            # 6250 dst nodes per core
WIN = 128
NW = (NLOC + WIN - 1) // WIN  # 49 windows
NPAD = NW * WIN               # 6272 padded local nodes
TROWS = NCORES * NPAD         # 50176 all-gathered table rows
CH = 512
N_ITERS = 3

LAST_EXEC_NS = None


def _chunks():
    out = []
    c = 0
    while c < NPAD:
        cn = min(CH, NPAD - c)
        out.append((c, cn))
        c += cn
    return out


def _build(sched, T):
    nc = bass.Bass("TRN2", num_devices=NCORES)

    def din(name, shape, dt=F32):
        return nc.dram_tensor(name, shape, dt, kind="ExternalInput").ap()

    xT_d = din("xT", [16, NPAD])
    heT_d = din("heT", [33, T * 128])
    idx_d = din("idx", [128, T], I32)
    aux_d = din("aux", [128, 2 * T + 128])
    ident_d = din("ident", [16, 16])
    we2k_d = din("we2k", [33, 256])
    wroot_d = din("wroot", [16, 16])
    wlin0_d = din("wlin0", [16, 16])
    blin0_d = din("blin0", [16, 1])
    bconv_d = din("bconv", [16, 1])
    wih_d = din("wih", [16, 48])
    whh_d = din("whh", [16, 48])
    br_d = din("br", [16, 1])
    bz_d = din("bz", [16, 1])
    bin_d = din("bin", [16, 1])
    bhn_d = din("bhn", [16, 1])
    wlin1_d = din("wlin1", [16, 4])
    blin1_d = din("blin1", [4, 1])
    wup_d = din("wup", [4, 16])
    bup_d = din("bup", [16, 1])
    em_d = din("em", [16, NPAD])
    ub_d = din("ub", [16, RANK])
    vb_d = din("vb", [RANK, 16])
    ua_d = din("ua", [4, RANK])
    va_d = din("va", [RANK, 4])
    wdown_d = din("wdown", [16, 4])
    bdown_d = din("bdown", [4, 1])
    wedge_d = din("wedge", [4, 1])
    wline_d = din("wline", [4, 4])
    bline_d = din("bline", [4, 1])
    oout_d = nc.dram_tensor("oout", [NPAD, 4], F32, kind="ExternalOutput").ap()

    chunks = _chunks()

    with tile.TileContext(nc) as tc:
        with tc.tile_pool(name="const", bufs=1) as cp, \
             tc.tile_pool(name="state", bufs=1) as sp, \
             tc.tile_pool(name="dram", bufs=1, space="DRAM") as dp:

            def cload(ap_d, shape, dt=F32, tag=None):
                t = cp.tile(shape, dt, tag=tag or ap_d.name, name=(tag or ap_d.name) + "_s")
                nc.sync.dma_start(t[:], ap_d[:])
                return t

            idx_s = cload(idx_d, [128, T], I32)
            aux_s = cload(aux_d, [128, 2 * T + 128])
            ident_s = cload(ident_d, [16, 16])
            we2k_s = cload(we2k_d, [33, 256])
            wroot_s = cload(wroot_d, [16, 16])
            wlin0_s = cload(wlin0_d, [16, 16])
            blin0_s = cload(blin0_d, [16, 1])
            bconv_s = cload(bconv_d, [16, 1])
            wih_s = cload(wih_d, [16, 48])
            whh_s = cload(whh_d, [16, 48])
            br_s = cload(br_d, [16, 1])
            bz_s = cload(bz_d, [16, 1])
            bin_s = cload(bin_d, [16, 1])
            bhn_s = cload(bhn_d, [16, 1])
            wlin1_s = cload(wlin1_d, [16, 4])
            blin1_s = cload(blin1_d, [4, 1])
            wup_s = cload(wup_d, [4, 16])
            bup_s = cload(bup_d, [16, 1])
            ub_s = cload(ub_d, [16, RANK])
            ua_s = cload(ua_d, [4, RANK])
            wdown_s = cload(wdown_d, [16, 4])
            bdown_s = cload(bdown_d, [4, 1])
            wedge_s = cload(wedge_d, [4, 1])
            wline_s = cload(wline_d, [4, 4])
            bline_s = cload(bline_d, [4, 1])

            vb_s = cp.tile([128, 4, 16], F32, tag="vb", name="vb_s")
            va_s = cp.tile([128, 4, 4], F32, tag="va", name="va_s")
            for r in range(4):
                nc.sync.dma_start(vb_s[:, r:r + 1, :].squeeze(1), vb_d[r * 128:(r + 1) * 128, :])
                nc.sync.dma_start(va_s[:, r:r + 1, :].squeeze(1), va_d[r * 128:(r + 1) * 128, :])

            stA = sp.tile([16, NPAD], F32, tag="stA", name="stA")
            stB = sp.tile([16, NPAD], F32, tag="stB", name="stB")

            bounce = dp.tile([NPAD, 16], F32, tag="bounce", name="bounce")
            table = dp.tile([TROWS, 16], F32, tag="table", name="table")

            # ---- lin0: st = relu(x @ W_lin0 + b_lin0), transposed layout ----
            with tc.tile_pool(name="initp", bufs=1) as ip, \
                 tc.tile_pool(name="initps", bufs=2, space="PSUM") as ips:
                xT_s = ip.tile([16, NPAD], F32, tag="xT", name="xT_s")
                nc.sync.dma_start(xT_s[:], xT_d[:])
                for (c0, cn) in chunks:
                    pl = ips.tile([16, cn], F32, name="pl")
                    nc.tensor.matmul(out=pl[:], lhsT=wlin0_s[:],
                                     rhs=xT_s[:, c0:c0 + cn],
                                     start=True, stop=True)
                    nc.scalar.activation(out=stA[:, c0:c0 + cn], in_=pl[:],
                                         func=AF.Relu, bias=blin0_s[:, 0:1])

            # ---- 3 message-passing + GRU iterations ----
            with tc.tile_pool(name="gat", bufs=1) as gp, \
                 tc.tile_pool(name="hep", bufs=2) as hp, \
                 tc.tile_pool(name="mtp", bufs=1) as mp, \
                 tc.tile_pool(name="edge_sb", bufs=3) as esb, \
                 tc.tile_pool(name="gru_sb", bufs=3) as gsb, \
                 tc.tile_pool(name="stage_sb", bufs=1) as stp, \
                 tc.tile_pool(name="we_ps", bufs=2, space="PSUM") as wps_p, \
                 tc.tile_pool(name="agg_ps", bufs=1, space="PSUM") as agg_p, \
                 tc.tile_pool(name="tp_ps", bufs=2, space="PSUM") as tp_p, \
                 tc.tile_pool(name="gru_ps", bufs=3, space="PSUM") as gru_p:

                G = gp.tile([128, T, 16], F32, tag="G", name="G")
                mT_s = mp.tile([16, NPAD], F32, tag="mT", name="mT_s")
                stage = stp.tile([128, NW, 16], F32, tag="stage", name="stage")

                st, nxt = stA, stB
                for it in range(N_ITERS):
                    # transpose state to row-major and publish via AllGather
                    for w in range(NW):
                        pt = tp_p.tile([128, 16], F32, name="pt")
                        nc.tensor.transpose(out=pt[:], in_=st[:, w * 128:(w + 1) * 128],
                                            identity=ident_s[:])
                        nc.scalar.activation(out=stage[:, w:w + 1, :].squeeze(1),
                                             in_=pt[:], func=AF.Copy)
                    nc.sync.dma_start(bounce.rearrange("(w p) d -> p w d", p=128), stage[:])
                    nc.gpsimd.collective_compute(
                        "AllGather", OP.bypass,
                        replica_groups=[list(range(NCORES))],
                        ins=[bounce.opt()], outs=[table.opt()],
                    )
                    # gather source-node states for every (padded) edge,
                    # one [128,1]-offset indirect DMA per tile (the only
                    # offset layout the unroller handles correctly)
                    for t in range(T):
                        nc.gpsimd.indirect_dma_start(
                            out=G[:, t:t + 1, :].squeeze(1), out_offset=None,
                            in_=table[:],
                            in_offset=bass.IndirectOffsetOnAxis(
                                ap=idx_s[:, t:t + 1], axis=0),
                        )

                    # edge phase: We matmul, per-edge contraction, scatter-matmul
                    for (w, t0, nt) in sched:
                        if nt > 0:
                            he_w = hp.tile([33, nt * 128], F32, name="he_w")
                            nc.sync.dma_start(he_w[:], heT_d[:, t0 * 128:(t0 + nt) * 128])
                        agg = agg_p.tile([16, 128], F32, name="agg")
                        for tl in range(nt):
                            t = t0 + tl
                            wps = wps_p.tile([128, 256], F32, name="wps")
                            nc.tensor.matmul(out=wps[:],
                                             lhsT=he_w[:, tl * 128:(tl + 1) * 128],
                                             rhs=we2k_s[:],
                                             start=True, stop=True)
                            sel = esb.tile([128, 128], F32, tag="sel", name="sel")
                            nc.vector.tensor_scalar(out=sel[:], in0=aux_s[:, 2 * T:2 * T + 128],
                                                    scalar1=aux_s[:, t:t + 1],
                                                    scalar2=aux_s[:, T + t:T + t + 1],
                                                    op0=OP.is_equal, op1=OP.mult)
                            prod = esb.tile([128, 16, 16], F32, tag="prod", name="prod")
                            nc.vector.tensor_tensor(
                                out=prod[:],
                                in0=wps[:].rearrange("p (k d) -> p k d", d=16),
                                in1=G[:, t:t + 1, :].to_broadcast([128, 16, 16]),
                                op=OP.mult)
                            mt = esb.tile([128, 16], F32, tag="mt", name="mt")
                            nc.vector.tensor_reduce(out=mt[:], in_=prod[:],
                                                    axis=AX.X, op=OP.add)
                            nc.tensor.matmul(out=agg[:], lhsT=mt[:], rhs=sel[:],
                                             start=(tl == 0), stop=False)
                        nc.tensor.matmul(out=agg[:], lhsT=wroot_s[:],
                                         rhs=st[:, w * 128:(w + 1) * 128],
                                         start=(nt == 0), stop=True)
                        nc.scalar.activation(out=mT_s[:, w * 128:(w + 1) * 128], in_=agg[:],
                                             func=AF.Relu, bias=bconv_s[:, 0:1])

                    # GRU: nxt = (1-z)*n + z*st, gates from mT_s (input) and st (hidden)
                    for (c0, cn) in chunks:
                        msl = mT_s[:, c0:c0 + cn]
                        ssl = st[:, c0:c0 + cn]
                        pr = gru_p.tile([16, cn], F32, tag="pg", name="pr")
                        nc.tensor.matmul(out=pr[:], lhsT=wih_s[:, 0:16],
                                         rhs=msl, start=True, stop=False)
                        nc.tensor.matmul(out=pr[:], lhsT=whh_s[:, 0:16],
                                         rhs=ssl, start=False, stop=True)
                        r = gsb.tile([16, cn], F32, tag="r", name="r")
                        nc.scalar.activation(out=r[:], in_=pr[:], func=AF.Sigmoid,
                                             bias=br_s[:, 0:1])
                        pz = gru_p.tile([16, cn], F32, tag="pg", name="pz")
                        nc.tensor.matmul(out=pz[:], lhsT=wih_s[:, 16:32],
                                         rhs=msl, start=True, stop=False)
                        nc.tensor.matmul(out=pz[:], lhsT=whh_s[:, 16:32],
                                         rhs=ssl, start=False, stop=True)
                        z = gsb.tile([16, cn], F32, tag="z", name="z")
                        nc.scalar.activation(out=z[:], in_=pz[:], func=AF.Sigmoid,
                                             bias=bz_s[:, 0:1])
                        pgn = gru_p.tile([16, cn], F32, tag="pg", name="pgn")
                        nc.tensor.matmul(out=pgn[:], lhsT=wih_s[:, 32:48],
                                         rhs=msl, start=True, stop=True)
                        phn = gru_p.tile([16, cn], F32, tag="pg", name="phn")
                        nc.tensor.matmul(out=phn[:], lhsT=whh_s[:, 32:48],
                                         rhs=ssl, start=True, stop=True)
                        hn = gsb.tile([16, cn], F32, tag="hn", name="hn")
                        nc.vector.tensor_scalar(out=hn[:], in0=phn[:],
                                                scalar1=bhn_s[:, 0:1], scalar2=None,
                                                op0=OP.add)
                        rhn = gsb.tile([16, cn], F32, tag="rhn", name="rhn")
                        nc.vector.tensor_tensor(out=rhn[:], in0=r[:], in1=hn[:], op=OP.mult)
                        npre = gsb.tile([16, cn], F32, tag="npre", name="npre")
                        nc.vector.tensor_tensor(out=npre[:], in0=pgn[:], in1=rhn[:], op=OP.add)
                        nn = gsb.tile([16, cn], F32, tag="nn", name="nn")
                        nc.scalar.activation(out=nn[:], in_=npre[:], func=AF.Tanh,
                                             bias=bin_s[:, 0:1])
                        dd = gsb.tile([16, cn], F32, tag="dd", name="dd")
                        nc.vector.tensor_tensor(out=dd[:], in0=ssl, in1=nn[:], op=OP.subtract)
                        zd = gsb.tile([16, cn], F32, tag="zd", name="zd")
                        nc.vector.tensor_tensor(out=zd[:], in0=z[:], in1=dd[:], op=OP.mult)
                        nc.vector.tensor_tensor(out=nxt[:, c0:c0 + cn], in0=nn[:], in1=zd[:],
                                                op=OP.add)
                    st, nxt = nxt, st

            # ---- final phase: edge beliefs + factor messages + log_softmax ----
            with tc.tile_pool(name="fin_sb", bufs=1) as fp, \
                 tc.tile_pool(name="fin_rot", bufs=2) as fr, \
                 tc.tile_pool(name="fin_sm", bufs=2) as fs4, \
                 tc.tile_pool(name="t1_ps", bufs=2, space="PSUM") as t1p, \
                 tc.tile_pool(name="acc_ps", bufs=2, space="PSUM") as accp, \
                 tc.tile_pool(name="sm_ps", bufs=2, space="PSUM") as smp:

                em_s = fp.tile([16, NPAD], F32, tag="em", name="em_s")
                nc.sync.dma_start(em_s[:], em_d[:])
                oeT_s = fp.tile([4, NPAD], F32, tag="oeT", name="oeT_s")
                oeF_s = fp.tile([4, NPAD], F32, tag="oeF", name="oeF_s")

                for (c0, cn) in chunks:
                    po = smp.tile([4, cn], F32, tag="ps", name="po")
                    nc.tensor.matmul(out=po[:], lhsT=wlin1_s[:],
                                     rhs=st[:, c0:c0 + cn],
                                     start=True, stop=True)
                    nc.scalar.activation(out=oeT_s[:, c0:c0 + cn], in_=po[:],
                                         func=AF.Relu, bias=blin1_s[:, 0:1])

                for (c0, cn) in chunks:
                    sl = slice(c0, c0 + cn)
                    # combine: where(ev_mask, oe @ W_up + b_up, st)
                    pu = smp.tile([16, cn], F32, tag="ps", name="pu")
                    nc.tensor.matmul(out=pu[:], lhsT=wup_s[:],
                                     rhs=oeT_s[:, sl], start=True, stop=True)
                    upb = fr.tile([16, cn], F32, tag="upb", name="upb")
                    nc.vector.tensor_scalar(out=upb[:], in0=pu[:], scalar1=bup_s[:, 0:1],
                                            scalar2=None, op0=OP.add)
                    d_ = fr.tile([16, cn], F32, tag="d_", name="d_")
                    nc.vector.tensor_tensor(out=d_[:], in0=upb[:], in1=st[:, sl],
                                            op=OP.subtract)
                    md = fr.tile([16, cn], F32, tag="md", name="md")
                    nc.vector.tensor_tensor(out=md[:], in0=em_s[:, sl], in1=d_[:], op=OP.mult)
                    comb = fr.tile([16, cn], F32, tag="comb", name="comb")
                    nc.vector.tensor_tensor(out=comb[:], in0=st[:, sl], in1=md[:], op=OP.add)

                    # msg_B = relu((comb @ U_B) @ V_B); mteB = msg_B @ W_down + b_down
                    accB = accp.tile([16, cn], F32, tag="acc", name="accB")
                    for r4 in range(4):
                        t1 = t1p.tile([128, cn], F32, tag="t1", name="t1")
                        nc.tensor.matmul(out=t1[:],
                                         lhsT=ub_s[:, r4 * 128:(r4 + 1) * 128],
                                         rhs=comb[:], start=True, stop=True)
                        t1s = fr.tile([128, cn], F32, tag="t1s", name="t1s")
                        nc.scalar.activation(out=t1s[:], in_=t1[:], func=AF.Copy)
                        nc.tensor.matmul(out=accB[:],
                                         lhsT=vb_s[:, r4:r4 + 1, :].squeeze(1),
                                         rhs=t1s[:],
                                         start=(r4 == 0), stop=(r4 == 3))
                    msgB = fr.tile([16, cn], F32, tag="msgB", name="msgB")
                    nc.scalar.activation(out=msgB[:], in_=accB[:], func=AF.Relu)
                    pdn = smp.tile([4, cn], F32, tag="ps", name="pdn")
                    nc.tensor.matmul(out=pdn[:], lhsT=wdown_s[:],
                                     rhs=msgB[:], start=True, stop=True)
                    mteB = fs4.tile([4, cn], F32, tag="mteB", name="mteB")
                    nc.vector.tensor_scalar(out=mteB[:], in0=pdn[:], scalar1=bdown_s[:, 0:1],
                                            scalar2=None, op0=OP.add)

                    # mteA = relu((oe @ U_A) @ V_A)
                    accA = accp.tile([4, cn], F32, tag="acc", name="accA")
                    for r4 in range(4):
                        t1 = t1p.tile([128, cn], F32, tag="t1", name="t1a")
                        nc.tensor.matmul(out=t1[:],
                                         lhsT=ua_s[:, r4 * 128:(r4 + 1) * 128],
                                         rhs=oeT_s[:, sl], start=True, stop=True)
                        t1s = fr.tile([128, cn], F32, tag="t1s", name="t1sa")
                        nc.scalar.activation(out=t1s[:], in_=t1[:], func=AF.Copy)
                        nc.tensor.matmul(out=accA[:],
                                         lhsT=va_s[:, r4:r4 + 1, :].squeeze(1),
                                         rhs=t1s[:],
                                         start=(r4 == 0), stop=(r4 == 3))
                    mteA = fs4.tile([4, cn], F32, tag="mteA", name="mteA")
                    nc.scalar.activation(out=mteA[:], in_=accA[:], func=AF.Relu)

                    # oeF = oeT + relu((w_edge * (mteA*mteB)) @ W_line + b_line)
                    ce = fs4.tile([4, cn], F32, tag="ce", name="ce")
                    nc.vector.tensor_tensor(out=ce[:], in0=mteA[:], in1=mteB[:], op=OP.mult)
                    sce = fs4.tile([4, cn], F32, tag="sce", name="sce")
                    nc.vector.tensor_scalar(out=sce[:], in0=ce[:], scalar1=wedge_s[:, 0:1],
                                            scalar2=None, op0=OP.mult)
                    pline = smp.tile([4, cn], F32, tag="ps", name="pline")
                    nc.tensor.matmul(out=pline[:], lhsT=wline_s[:],
                                     rhs=sce[:], start=True, stop=True)
                    adde = fs4.tile([4, cn], F32, tag="adde", name="adde")
                    nc.scalar.activation(out=adde[:], in_=pline[:], func=AF.Relu,
                                         bias=bline_s[:, 0:1])
                    nc.vector.tensor_tensor(out=oeF_s[:, sl], in0=oeT_s[:, sl], in1=adde[:],
                                            op=OP.add)

                # log_softmax over bond dim: transpose to row-major then reduce
                rs_all = fp.tile([128, NW, 4], F32, tag="rs", name="rs_all")
                for w in range(NW):
                    pt = smp.tile([128, 4], F32, tag="ps", name="ptf")
                    nc.tensor.transpose(out=pt[:], in_=oeF_s[:, w * 128:(w + 1) * 128],
                                        identity=ident_s[0:4, 0:4])
                    nc.scalar.activation(out=rs_all[:, w:w + 1, :].squeeze(1), in_=pt[:],
                                         func=AF.Copy)
                mx = fp.tile([128, NW], F32, tag="mx", name="mx")
                nc.vector.tensor_reduce(out=mx[:], in_=rs_all[:], axis=AX.X, op=OP.max)
                sub = fp.tile([128, NW, 4], F32, tag="sub", name="sub")
                nc.vector.tensor_tensor(out=sub[:], in0=rs_all[:],
                                        in1=mx[:].unsqueeze(2).to_broadcast([128, NW, 4]),
                                        op=OP.subtract)
                ex = fp.tile([128, NW, 4], F32, tag="ex", name="ex")
                nc.scalar.activation(out=ex[:], in_=sub[:], func=AF.Exp)
                sm = fp.tile([128, NW], F32, tag="sm", name="sm")
                nc.vector.tensor_reduce(out=sm[:], in_=ex[:], axis=AX.X, op=OP.add)
                ls = fp.tile([128, NW], F32, tag="ls", name="ls")
                nc.scalar.activation(out=ls[:], in_=sm[:], func=AF.Ln)
                res = fp.tile([128, NW, 4], F32, tag="res", name="res")
                nc.vector.tensor_tensor(out=res[:], in0=sub[:],
                                        in1=ls[:].unsqueeze(2).to_broadcast([128, NW, 4]),
                                        op=OP.subtract)
                nc.sync.dma_start(oout_d.rearrange("(w p) d -> p w d", p=128), res[:])

    import bass_rust as _bass_rust
    _bass_rust.move_matmul_waits_to_ldweights(nc.m)
    _bass_rust.generate_event_semaphores(nc)
    return nc


def _time_pjrt(nc, in_maps, n_cores, reps=50):
    import time
    import jax
    from jax.sharding import Mesh, PartitionSpec, NamedSharding
    from jax.experimental.shard_map import shard_map
    from concourse import bass2jax as b2j
    from concourse import mybir

    b2j.install_neuronx_cc_hook()
    partition_name = nc.partition_id_tensor.name if nc.partition_id_tensor else None
    in_names, out_names, out_avals, zero_outs = [], [], [], []
    for alloc in nc.m.functions[0].allocations:
        if not isinstance(alloc, mybir.MemoryLocationSet):
            continue
        name = alloc.memorylocations[0].name
        if alloc.kind == "ExternalInput":
            if name != partition_name:
                in_names.append(name)
        elif alloc.kind == "ExternalOutput":
            shape = tuple(alloc.tensor_shape)
            dtype = mybir.dt.np(alloc.dtype)
            out_names.append(name)
            out_avals.append(jax.core.ShapedArray(shape, dtype))
            zero_outs.append(np.zeros(shape, dtype))
    n_params = len(in_names)
    n_outs = len(out_avals)
    in_names_all = list(in_names) + list(out_names)
    if partition_name is not None:
        in_names_all.append(partition_name)

    def _body(*args):
        operands = list(args)
        if partition_name is not None:
            operands.append(b2j.partition_id_tensor())
        outs = b2j._bass_exec_p.bind(
            *operands,
            out_avals=tuple(out_avals),
            in_names=tuple(in_names_all),
            out_names=tuple(out_names),
            lowering_input_output_aliases=(),
            sim_require_finite=True,
            sim_require_nnan=True,
            nc=nc,
        )
        return tuple(outs)

    devices = jax.devices()[:n_cores]
    mesh = Mesh(np.asarray(devices), ("core",))
    in_specs = (PartitionSpec("core"),) * (n_params + n_outs)
    out_specs = (PartitionSpec("core"),) * n_outs
    sharded = jax.jit(
        shard_map(_body, mesh=mesh, in_specs=in_specs,
                  out_specs=out_specs, check_rep=False),
        keep_unused=True)
    concat_in = [
        np.concatenate([np.asarray(in_maps[c][nm]) for c in range(n_cores)], axis=0)
        for nm in in_names]
    concat_zeros = [np.zeros((n_cores * z.shape[0], *z.shape[1:]), z.dtype)
                    for z in zero_outs]
    shd = NamedSharding(mesh, PartitionSpec("core"))
    dev_in = [jax.device_put(a, shd) for a in concat_in]
    dev_zeros = [jax.device_put(a, shd) for a in concat_zeros]
    outs = sharded(*dev_in, *dev_zeros)
    jax.block_until_ready(outs)
    t0 = time.perf_counter()
    for _ in range(reps):
        outs = sharded(*dev_in, *dev_zeros)
    jax.block_until_ready(outs)
    t1 = time.perf_counter()
    return (t1 - t0) / reps * 1e9


def _prep(inputs):
    x = np.ascontiguousarray(np.asarray(inputs["x"], np.float32))
    node_type = np.asarray(inputs["node_type"]).astype(np.int64)
    ei = np.asarray(inputs["edge_index"]).astype(np.int64)
    ea = np.ascontiguousarray(np.asarray(inputs["edge_attr"], np.float32))
    W = {k: np.asarray(v, np.float32) for k, v in inputs.items()
         if k not in ("x", "node_type", "edge_index", "edge_attr")}

    src, dst = ei[0], ei[1]
    he = np.maximum(ea @ W["W_e1"] + W["b_e1"], 0.0).astype(np.float32)  # [E,32]
    deg = np.bincount(dst, minlength=N).astype(np.float32)
    invdeg = (1.0 / np.maximum(deg, 1.0)).astype(np.float32)
    order = np.argsort(dst, kind="stable")
    src_s = src[order]
    dst_s = dst[order]
    he_s = he[order]

    # identical schedule across cores: tiles per window = max over cores
    lo_all = np.empty((NCORES, NW), np.int64)
    hi_all = np.empty((NCORES, NW), np.int64)
    for c in range(NCORES):
        for w in range(NW):
            lo_all[c, w] = c * NLOC + w * WIN
            hi_all[c, w] = c * NLOC + min((w + 1) * WIN, NLOC)
    e_lo = np.searchsorted(dst_s, lo_all.ravel()).reshape(NCORES, NW)
    e_hi = np.searchsorted(dst_s, hi_all.ravel()).reshape(NCORES, NW)
    counts = e_hi - e_lo
    tiles_w = np.maximum((counts.max(axis=0) + 127) // 128, 0).astype(np.int64)
    T = int(tiles_w.sum())
    sched = []
    t0 = 0
    for w in range(NW):
        sched.append((w, t0, int(tiles_w[w])))
        t0 += int(tiles_w[w])

    # k-major permutation for the edge-conditioned weight matrix
    J = np.arange(256).reshape(16, 16).T.reshape(-1)
    we2k = np.concatenate([W["W_e2"][:, J], W["b_e2"][J][None, :]], axis=0)
    we2k = np.ascontiguousarray(we2k, np.float32)  # [33,256]

    iota = np.ascontiguousarray(
        np.broadcast_to(np.arange(128, dtype=np.float32), (128, 128)))
    common = {
        "ident": np.eye(16, dtype=np.float32),
        "we2k": we2k,
        "wroot": np.ascontiguousarray(W["W_root"]),
        "wlin0": np.ascontiguousarray(W["W_lin0"]),
        "blin0": W["b_lin0"].reshape(16, 1).copy(),
        "bconv": W["b_conv"].reshape(16, 1).copy(),
        "wih": np.ascontiguousarray(W["W_ih"].T),   # [16,48]
        "whh": np.ascontiguousarray(W["W_hh"].T),   # [16,48]
        "br": (W["b_ih"][0:16] + W["b_hh"][0:16]).reshape(16, 1).copy(),
        "bz": (W["b_ih"][16:32] + W["b_hh"][16:32]).reshape(16, 1).copy(),
        "bin": W["b_ih"][32:48].reshape(16, 1).copy(),
        "bhn": W["b_hh"][32:48].reshape(16, 1).copy(),
        "wlin1": np.ascontiguousarray(W["W_lin1"]),
        "blin1": W["b_lin1"].reshape(4, 1).copy(),
        "wup": np.ascontiguousarray(W["W_up"]),
        "bup": W["b_up"].reshape(16, 1).copy(),
        "ub": np.ascontiguousarray(W["U_B"]),
        "vb": np.ascontiguousarray(W["V_B"]),
        "ua": np.ascontiguousarray(W["U_A"]),
        "va": np.ascontiguousarray(W["V_A"]),
        "wdown": np.ascontiguousarray(W["W_down"]),
        "bdown": W["b_down"].reshape(4, 1).copy(),
        "wedge": W["w_edge"].reshape(4, 1).copy(),
        "wline": np.ascontiguousarray(W["W_line"]),
        "bline": W["b_line"].reshape(4, 1).copy(),
    }

    in_maps = []
    for c in range(NCORES):
        slots = T * 128
        src_pad = np.zeros(slots, np.int64)
        dstl = np.full(slots, -1.0, np.float32)
        ivd = np.zeros(slots, np.float32)
        hepad = np.zeros((slots, 33), np.float32)
        hepad[:, 32] = 1.0
        for (w, tw0, nt) in sched:
            e0, e1 = int(e_lo[c, w]), int(e_hi[c, w])
            k = e1 - e0
            base = tw0 * 128
            if k > 0:
                src_pad[base:base + k] = src_s[e0:e1]
                dstl[base:base + k] = (dst_s[e0:e1] - lo_all[c, w]).astype(np.float32)
                ivd[base:base + k] = invdeg[dst_s[e0:e1]]
                hepad[base:base + k, 0:32] = he_s[e0:e1]
        idx = ((src_pad // NLOC) * NPAD + (src_pad % NLOC)).astype(np.int32)
        xT = np.zeros((16, NPAD), np.float32)
        xT[:, :NLOC] = x[c * NLOC:(c + 1) * NLOC].T
        em = np.zeros((16, NPAD), np.float32)
        em[:, :NLOC] = (node_type[c * NLOC:(c + 1) * NLOC] == 2).astype(np.float32)[None, :]
        m = dict(common)
        m.update({
            "xT": xT,
            "heT": np.ascontiguousarray(hepad.T),                    # [33, T*128]
            "idx": np.ascontiguousarray(idx.reshape(T, 128).T),      # [128, T]
            "aux": np.ascontiguousarray(np.concatenate(
                [dstl.reshape(T, 128).T, ivd.reshape(T, 128).T, iota], axis=1)),
            "em": em,
        })
        in_maps.append(m)
    return sched, T, in_maps


def kernel(**inputs):
    global LAST_EXEC_NS
    sched, T, in_maps = _prep(inputs)
    nc = _build(sched, T)
    results = run_bass_kernel_spmd(nc, in_maps, core_ids=list(range(NCORES)), trace=False)
    LAST_EXEC_NS = results.exec_time_ns
    if os.environ.get("KTRACE") == "1":
        try:
            LAST_EXEC_NS = _time_pjrt(nc, in_maps, NCORES)
        except Exception as e:
            print("timing failed:", e)

    outs = results.results
    parts = []
    for c in range(NCORES):
        r = outs[c]
        arr = r["oout"] if isinstance(r, dict) else r[0]
        parts.append(np.asarray(arr)[:NLOC])
    return np.ascontiguousarray(np.concatenate(parts, axis=0).astype(np.float32))
